# revision 1
# baseline (speedup 1.0000x reference)
"""Trainium2 Bass kernel for PhysicsInformedCtxLiquidNetwork (1024-step liquid NN).

Self-contained: hardcodes shapes/sharding. Accepts FULL inputs, returns FULL output.

Strategy (data-parallel over batch, 8 cores x 32 batch):
  - State h~ = h/DT kept in SBUF [B,512]; update h~' = a*h~ + f with scalar
    a = 1 - DT/softplus(tau); clip is provably inactive for softplus(tau)<=10;
    final LayerNorm is scale-invariant so LN(h~) == LN(h).
  - Per step: 5 accumulating matmuls (float32r) build z[B,512] in PSUM
    (4 K=128 h-chunks from a PE-transposed copy of the state + one K=3 chunk
    carrying [x_t; 1] against [in_w; in_b]); 5 N=1 matmuls against
    precomputed row-sum columns give S1 = sum(z).
  - Stats: ACT Square with accum_out -> Q = sum(z^2); variance from Q and S1;
    rsqrt via int32 bit-hack + 2 Newton iterations on DVE (ACT Rsqrt is
    banned and Sqrt lives in a conflicting activation-table set).
  - f = tanh(z*rstd + bias2) in one ACT op (per-partition scale/bias).
  - Update on GPSIMD scalar_tensor_tensor; state re-transposed with 4 PE
    transpose ops + one DVE PSUM->SBUF copy.
"""

import math
import numpy as np
from contextlib import ExitStack

import concourse.bass as bass
import concourse.bacc as bacc
import concourse.tile as tile
from concourse import mybir
from concourse.bass_utils import run_bass_kernel_spmd

F32 = mybir.dt.float32
F32R = mybir.dt.float32r
I32 = mybir.dt.int32
AF = mybir.ActivationFunctionType
OP = mybir.AluOpType

HIDDEN = 512
INPUT = 2
CTX = 6
NAPP = 20
DT = 0.1
S = 1024
B_FULL = 256
N_CORES = 8
BL = B_FULL // N_CORES  # 32 per core
EPS = 1e-5
MAGIC = 0x5F3759DF

_BUILD_CACHE = {}


def _emit_rstd(nc, tiny, ve, tag):
    """rstd = 1/sqrt(ve) via bit-hack seed + 2 Newton iters. Returns rstd AP."""
    s = tiny.tile([BL, 1], F32, tag=f"s_{tag}")
    t1 = tiny.tile([BL, 1], F32, tag=f"t1_{tag}")
    # seed bits: (MAGIC+1) + ~(ve_i >> 1)  == MAGIC - (ve_i >> 1)
    nc.vector.tensor_scalar(
        t1.bitcast(I32), ve.bitcast(I32), 1, -1, OP.arith_shift_right, OP.bitwise_xor
    )
    nc.vector.tensor_scalar(s.bitcast(I32), t1.bitcast(I32), MAGIC + 1, None, OP.add)
    q = tiny.tile([BL, 1], F32, tag=f"q_{tag}")
    w = tiny.tile([BL, 1], F32, tag=f"w_{tag}")
    for _ in range(2):
        nc.vector.tensor_mul(q, s, s)
        nc.vector.scalar_tensor_tensor(w, q, -0.5, ve, OP.mult, OP.mult)
        nc.vector.scalar_tensor_tensor(s, w, 1.5, s, OP.add, OP.mult)
    return s


def _emit_stats(nc, tiny, zs_src, q_src, eps_tile, tag):
    """From S1 (zs_src [BL,1]) and Q (q_src [BL,1]): negmu, ve. Returns (negmu, ve)."""
    negmu = tiny.tile([BL, 1], F32, tag=f"negmu_{tag}")
    nc.vector.tensor_scalar(negmu, zs_src, -1.0 / HIDDEN, None, OP.mult)
    m2e = tiny.tile([BL, 1], F32, tag=f"m2e_{tag}")
    # m2e = negmu*negmu - eps
    nc.vector.scalar_tensor_tensor(m2e, negmu, negmu, eps_tile, OP.mult, OP.subtract)
    ve = tiny.tile([BL, 1], F32, tag=f"ve_{tag}")
    # ve = Q/512 - m2e  (= var + eps)
    nc.vector.scalar_tensor_tensor(ve, q_src, 1.0 / HIDDEN, m2e, OP.mult, OP.subtract)
    return negmu, ve


def _emit_transpose(nc, hT_ps, hT_sb, src, ident):
    """src [BL,512] SBUF -> hT_sb [128, 4*BL] SBUF via 4 PE transposes + copy."""
    for c in range(4):
        nc.tensor.transpose(
            hT_ps[:, c * BL : (c + 1) * BL], src[:, c * 128 : (c + 1) * 128], ident
        )
    nc.vector.tensor_copy(hT_sb, hT_ps)


def _build(n_steps, gen_flags, a_val):
    key = (n_steps, gen_flags, float(a_val))
    if key in _BUILD_CACHE:
        return _BUILD_CACHE[key]
    need_intra_aff, need_tau_vec, need_clip, need_norm_aff = gen_flags

    nc = bacc.Bacc("TRN2", target_bir_lowering=False, debug=False)

    xa_d = nc.dram_tensor("xa", [INPUT + 1, n_steps * BL], F32R, kind="ExternalInput")
    wrec_d = nc.dram_tensor("wrec", [128, 4 * 514], F32R, kind="ExternalInput")
    wx_d = nc.dram_tensor("wx", [INPUT + 1, 514], F32R, kind="ExternalInput")
    ctxa_d = nc.dram_tensor("ctxa", [CTX + 1, BL], F32, kind="ExternalInput")
    cw1_d = nc.dram_tensor("cw1", [CTX + 1, 32], F32, kind="ExternalInput")
    cw2_d = nc.dram_tensor("cw2", [33, HIDDEN], F32, kind="ExternalInput")
    hw_d = nc.dram_tensor("hw", [128, 4 * NAPP], F32R, kind="ExternalInput")
    hb_d = nc.dram_tensor("hb", [1, NAPP], F32, kind="ExternalInput")
    ident_d = nc.dram_tensor("ident", [32, 32], F32, kind="ExternalInput")
    # general-path per-element params, replicated to [BL, 512] on host
    if need_intra_aff:
        ig_d = nc.dram_tensor("ig_rep", [BL, HIDDEN], F32, kind="ExternalInput")
        ib_d = nc.dram_tensor("ib_rep", [BL, HIDDEN], F32, kind="ExternalInput")
    if need_tau_vec:
        av_d = nc.dram_tensor("a_rep", [BL, HIDDEN], F32, kind="ExternalInput")
    if need_norm_aff:
        ng_d = nc.dram_tensor("ng_rep", [BL, HIDDEN], F32, kind="ExternalInput")
        nb_d = nc.dram_tensor("nb_rep", [BL, HIDDEN], F32, kind="ExternalInput")
    out_d = nc.dram_tensor("out", [BL, NAPP], F32, kind="ExternalOutput")

    with tile.TileContext(nc) as tc, ExitStack() as ctx:
        const = ctx.enter_context(tc.tile_pool(name="const", bufs=1))
        state = ctx.enter_context(tc.tile_pool(name="state", bufs=2))
        work = ctx.enter_context(tc.tile_pool(name="work", bufs=2))
        tiny = ctx.enter_context(tc.tile_pool(name="tiny", bufs=2))
        psum = ctx.enter_context(tc.tile_pool(name="psum", bufs=2, space="PSUM"))
        psum_s = ctx.enter_context(tc.tile_pool(name="psum_s", bufs=2, space="PSUM"))
        psum_t = ctx.enter_context(tc.tile_pool(name="psum_t", bufs=2, space="PSUM"))

        # ---- load constants ----
        xa = const.tile([INPUT + 1, n_steps * BL], F32R)
        wrec = const.tile([128, 4 * 514], F32R)
        wx = const.tile([INPUT + 1, 514], F32R)
        ctxa = const.tile([CTX + 1, BL], F32)
        cw1 = const.tile([CTX + 1, 32], F32)
        cw2 = const.tile([33, HIDDEN], F32)
        hw = const.tile([128, 4 * NAPP], F32R)
        hb = const.tile([1, NAPP], F32)
        ident = const.tile([32, 32], F32)
        for sb, dr in ((xa, xa_d), (wrec, wrec_d), (wx, wx_d), (ctxa, ctxa_d),
                       (cw1, cw1_d), (cw2, cw2_d), (hw, hw_d), (hb, hb_d),
                       (ident, ident_d)):
            nc.sync.dma_start(sb[:], dr[:])
        rep = {}
        if need_intra_aff:
            rep["ig"] = const.tile([BL, HIDDEN], F32)
            rep["ib"] = const.tile([BL, HIDDEN], F32)
            nc.sync.dma_start(rep["ig"][:], ig_d[:])
            nc.sync.dma_start(rep["ib"][:], ib_d[:])
        if need_tau_vec:
            rep["av"] = const.tile([BL, HIDDEN], F32)
            nc.sync.dma_start(rep["av"][:], av_d[:])
        if need_norm_aff:
            rep["ng"] = const.tile([BL, HIDDEN], F32)
            rep["nb"] = const.tile([BL, HIDDEN], F32)
            nc.sync.dma_start(rep["ng"][:], ng_d[:])
            nc.sync.dma_start(rep["nb"][:], nb_d[:])
        eps_tile = const.tile([BL, 1], F32)
        nc.vector.memset(eps_tile, EPS)
        ones_row = const.tile([1, BL], F32)
        nc.vector.memset(ones_row, 1.0)

        # ---- h0 = tanh(relu([ctx,1]@cw1) @ cw2-aug); h~0 = h0/DT ----
        p1 = psum_s.tile([BL, 32], F32, tag="hT_ps")
        nc.tensor.matmul(p1, ctxa[:], cw1[:], start=True, stop=True)
        r1 = work.tile([BL, 32], F32, tag="r1")
        nc.scalar.activation(r1, p1, AF.Relu)
        r1t = work.tile([33, BL], F32, tag="r1t")
        nc.vector.transpose(r1t[0:32, :], r1[:, :])
        nc.vector.memset(r1t[32:33, :], 1.0)
        p2 = psum.tile([BL, HIDDEN], F32, tag="z")
        nc.tensor.matmul(p2, r1t[:], cw2[:], start=True, stop=True)
        th = work.tile([BL, HIDDEN], F32, tag="th")
        nc.scalar.activation(th, p2, AF.Tanh)
        hA = state.tile([BL, HIDDEN], F32, tag="hA")
        nc.vector.tensor_scalar(hA, th, 1.0 / DT, None, OP.mult)
        hT_ps = psum_s.tile([128, 4 * BL], F32, tag="hT_ps")
        hT = state.tile([128, 4 * BL], F32R, tag="hT")
        _emit_transpose(nc, hT_ps, hT, hA, ident)

        # ---- main loop ----
        for t in range(n_steps):
            z = psum.tile([BL, HIDDEN], F32, tag="z")
            zs = psum_t.tile([BL, 2], F32, tag="zs")
            xa_t = xa[:, t * BL : (t + 1) * BL]
            nc.tensor.matmul(z, xa_t, wx[:, 0:HIDDEN],
                             start=True, stop=False)
            nc.tensor.matmul(zs, xa_t, wx[:, HIDDEN:HIDDEN+2],
                             start=True, stop=False)
            for c in range(4):
                lhsT = hT[:, c * BL : (c + 1) * BL]
                nc.tensor.matmul(z, lhsT, wrec[:, c * 514 : c * 514 + 512],
                                 start=False, stop=(c == 3))
                nc.tensor.matmul(zs, lhsT, wrec[:, c * 514 + 512 : c * 514 + 514],
                                 start=False, stop=(c == 3))
            # Q = sum(z^2) via ACT square + accumulator
            sq = work.tile([BL, HIDDEN], F32, tag="sq")
            Q = tiny.tile([BL, 1], F32, tag="Q")
            nc.scalar.activation(sq, z, AF.Square, accum_out=Q)
            negmu, ve = _emit_stats(nc, tiny, zs[:, 0:1], Q, eps_tile, "m")
            rstd = _emit_rstd(nc, tiny, ve, "m")
            bias2 = tiny.tile([BL, 1], F32, tag="bias2")
            nc.vector.tensor_mul(bias2, negmu, rstd)
            f = work.tile([BL, HIDDEN], F32, tag="f")
            if not need_intra_aff:
                nc.scalar.activation(f, z, AF.Tanh, bias=bias2, scale=rstd)
            else:
                u = work.tile([BL, HIDDEN], F32, tag="u")
                nc.scalar.activation(u, z, AF.Identity, bias=bias2, scale=rstd)
                nc.vector.tensor_mul(u, u, rep["ig"])
                nc.vector.tensor_add(u, u, rep["ib"])
                nc.scalar.activation(f, u, AF.Tanh)
            hA_new = state.tile([BL, HIDDEN], F32, tag="hA")
            if not need_tau_vec:
                nc.vector.scalar_tensor_tensor(hA_new, hA, float(a_val), f, OP.mult, OP.add)
            else:
                nc.vector.tensor_mul(hA_new, hA, rep["av"])
                nc.vector.tensor_add(hA_new, hA_new, f)
            if need_clip:
                nc.vector.tensor_scalar(hA_new, hA_new, 10.0 / DT, -10.0 / DT,
                                        OP.min, OP.max)
            hA = hA_new
            if t != n_steps - 1:
                hT_ps = psum_s.tile([128, 4 * BL], F32, tag="hT_ps")
                hT = state.tile([128, 4 * BL], F32R, tag="hT")
                _emit_transpose(nc, hT_ps, hT, hA, ident)

        # ---- final LN + head ----
        S1h = tiny.tile([BL, 1], F32, tag="S1h")
        nc.vector.tensor_reduce(S1h, hA, mybir.AxisListType.X, OP.add)
        sqf = work.tile([BL, HIDDEN], F32, tag="sq")
        Qf = tiny.tile([BL, 1], F32, tag="Qf")
        nc.scalar.activation(sqf, hA, AF.Square, accum_out=Qf)
        negmu, ve = _emit_stats(nc, tiny, S1h, Qf, eps_tile, "f")
        rstd = _emit_rstd(nc, tiny, ve, "f")
        bias2 = tiny.tile([BL, 1], F32, tag="bias2f")
        nc.vector.tensor_mul(bias2, negmu, rstd)
        ln = work.tile([BL, HIDDEN], F32, tag="ln")
        nc.scalar.activation(ln, hA, AF.Identity, bias=bias2, scale=rstd)
        if need_norm_aff:
            nc.vector.tensor_mul(ln, ln, rep["ng"])
            nc.vector.tensor_add(ln, ln, rep["nb"])
        lnT_ps = psum_s.tile([128, 4 * BL], F32, tag="hT_ps")
        lnT = state.tile([128, 4 * BL], F32R, tag="hT")
        _emit_transpose(nc, lnT_ps, lnT, ln, ident)
        po = psum_s.tile([BL, NAPP], F32, tag="hT_ps")
        nc.tensor.matmul(po, ones_row[:], hb[:], start=True, stop=False)
        for c in range(4):
            nc.tensor.matmul(po, lnT[:, c * BL : (c + 1) * BL],
                             hw[:, c * NAPP : (c + 1) * NAPP],
                             start=False, stop=(c == 3))
        res = work.tile([BL, NAPP], F32, tag="res")
        nc.vector.tensor_copy(res, po)
        nc.sync.dma_start(out_d[:], res[:])

    nc.compile()
    _BUILD_CACHE[key] = nc
    return nc


def _softplus(v):
    return np.log1p(np.exp(-np.abs(v))) + np.maximum(v, 0.0)


def kernel(**inputs):
    inputs = {k: np.ascontiguousarray(np.asarray(v)) for k, v in inputs.items()}
    x = inputs["x"].astype(np.float32)
    ctxv = inputs["ctx"].astype(np.float32)
    rec_w = inputs["rec_w"].astype(np.float32)
    in_w = inputs["in_w"].astype(np.float32)
    in_b = inputs["in_b"].astype(np.float32)
    tau = inputs["tau"].astype(np.float32)
    intra_g, intra_b = inputs["intra_g"].astype(np.float32), inputs["intra_b"].astype(np.float32)
    norm_g, norm_b = inputs["norm_g"].astype(np.float32), inputs["norm_b"].astype(np.float32)
    head_w, head_b = inputs["head_w"].astype(np.float32), inputs["head_b"].astype(np.float32)
    ce_w1, ce_b1 = inputs["ce_w1"].astype(np.float32), inputs["ce_b1"].astype(np.float32)
    ce_w2, ce_b2 = inputs["ce_w2"].astype(np.float32), inputs["ce_b2"].astype(np.float32)

    B, S_in, _ = x.shape
    assert B == B_FULL, B

    tau_sp = _softplus(tau).astype(np.float32)
    a_vec = (np.float32(1.0) - np.float32(DT) / tau_sp).astype(np.float32)
    need_tau_vec = not bool(np.all(a_vec == a_vec[0]))
    need_clip = not bool(np.all(tau_sp <= 10.0) and np.all(tau_sp >= DT))
    need_intra_aff = not (np.all(intra_g == 1.0) and np.all(intra_b == 0.0))
    need_norm_aff = not (np.all(norm_g == 1.0) and np.all(norm_b == 0.0))
    gen_flags = (need_intra_aff, need_tau_vec, need_clip, need_norm_aff)
    a_val = float(a_vec[0])

    nc = _build(S_in, gen_flags, a_val)

    # ---- host-side constant prep ----
    Wd = (rec_w * np.float32(DT)).astype(np.float32)  # z = h~ @ (DT*W) + x@in_w + in_b
    wrec = np.zeros((128, 4 * 514), np.float32)
    for c in range(4):
        blk = Wd[c * 128 : (c + 1) * 128, :]
        wrec[:, c * 514 : c * 514 + 512] = blk
        wrec[:, c * 514 + 512] = blk.sum(axis=1)
    wx = np.zeros((INPUT + 1, 514), np.float32)
    wx[0:INPUT, 0:HIDDEN] = in_w
    wx[INPUT, 0:HIDDEN] = in_b
    wx[0:INPUT, HIDDEN] = in_w.sum(axis=1)
    wx[INPUT, HIDDEN] = in_b.sum()
    cw1 = np.concatenate([ce_w1, ce_b1[None, :]], axis=0).astype(np.float32)  # [7,32]
    cw2 = np.concatenate([ce_w2, ce_b2[None, :]], axis=0).astype(np.float32)  # [33,512]
    hw = np.zeros((128, 4 * NAPP), np.float32)
    for c in range(4):
        hw[:, c * NAPP : (c + 1) * NAPP] = head_w[c * 128 : (c + 1) * 128, :]
    hb = head_b[None, :].astype(np.float32)
    ident = np.eye(32, dtype=np.float32)

    xt = np.transpose(x, (2, 1, 0))  # [2, S, B]
    in_maps = []
    for core in range(N_CORES):
        sl = slice(core * BL, (core + 1) * BL)
        xa = np.ones((INPUT + 1, S_in * BL), np.float32)
        xa[0:INPUT] = xt[:, :, sl].reshape(INPUT, S_in * BL)
        ctxa = np.ones((CTX + 1, BL), np.float32)
        ctxa[0:CTX] = ctxv[sl].T
        m = {
            "xa": xa, "wrec": wrec, "wx": wx, "ctxa": ctxa,
            "cw1": cw1, "cw2": cw2, "hw": hw, "hb": hb, "ident": ident,
        }
        if need_intra_aff:
            m["ig_rep"] = np.broadcast_to(intra_g, (BL, HIDDEN)).copy()
            m["ib_rep"] = np.broadcast_to(intra_b, (BL, HIDDEN)).copy()
        if need_tau_vec:
            m["a_rep"] = np.broadcast_to(a_vec, (BL, HIDDEN)).copy()
        if need_norm_aff:
            m["ng_rep"] = np.broadcast_to(norm_g, (BL, HIDDEN)).copy()
            m["nb_rep"] = np.broadcast_to(norm_b, (BL, HIDDEN)).copy()
        in_maps.append(m)

    br = run_bass_kernel_spmd(nc, in_maps, core_ids=list(range(N_CORES)))
    out = np.concatenate([np.asarray(r["out"]) for r in br.results], axis=0)
    global _LAST_RUN
    _LAST_RUN = (nc, in_maps)
    return out.astype(np.float32)


_LAST_RUN = None


def profile_exec_time_ns():
    """Re-run the last kernel invocation with NTFF tracing; return exec ns."""
    if _LAST_RUN is None:
        return None
    nc, in_maps = _LAST_RUN
    br = run_bass_kernel_spmd(nc, in_maps, core_ids=list(range(N_CORES)), trace=True)
    return br.exec_time_ns

